# revision 23
# baseline (speedup 1.0000x reference)
"""Trainium2 Bass kernel for the DTFA (dual-attention SE + threshold
decomposition) module.

Math (per batch b):
  zt = SE(mean_T(x))            # [C, F]
  zf = SE(mean_F(x))            # [C, T]
  out1[t,f] = sum_c wf[c]*zf[c,t]*zt[c,f] + bf          (rank-C matmul)
  dcomp[k]  = where(out1 > thr_k, out1, 0), k=1..23
  out[c]    = (sum_k wf2[c,k]*dcomp[k] + bf2[c]) * x[c]

Sharding: pure data-parallel, 2 batches per core on 8 cores.

Pass 1 streams the input as [128t, 8c*256f] tiles (1 MB DMAs): T-sums via
PE ones-matmuls into [1, 2048] PSUM rows, F-sums via DVE tensor_reduce.
Pass 2 processes 1024-pixel block pairs (blocks 2i, 2i+1): a PE
broadcast-matmul replicates out1 into 2x(ones + 23 thresholds) x 2
batches rows ([112, 512] PSUM, bias folded via the ones row), one fused
DVE scalar_tensor_tensor forms (x > thr) * x, a block-diagonal [bf2|wf2]
matmul produces out2 for both batches at once, and a single DVE
tensor_tensor multiplies by the re-streamed input.
"""

import numpy as np

B, C, OC, T, F = 16, 64, 16, 256, 256
N_THR = 23
N_CORES = 8
BL = B // N_CORES  # local batches per core = 2
PIX = T * F        # 65536 per batch
NPAIR = 64         # pairs of adjacent 512-pix blocks (t-quads) per batch

_CACHE = {}


def _host_constants(w1, b1, w2, b2, wf, bf, wf2, bf2):
    f32 = np.float32
    c = {}
    # SE weights. lhsT layout [K, M]; fold the 1/256 mean scale into w1.
    c["w1Ts"] = np.ascontiguousarray(w1.T.astype(f32) / 256.0)          # [64, 16]
    c["w2T"] = np.ascontiguousarray(w2.T.astype(f32))                   # [16, 64]
    c["b1c"] = b1.astype(f32).reshape(OC, 1)
    c["b2c"] = b2.astype(f32).reshape(C, 1)
    c["wfcol"] = wf.astype(f32).reshape(C, 1)                            # [64, 1]
    bf_s = float(np.asarray(bf).reshape(-1)[0])

    # Broadcast matmul weights: xB[m, n] = sum_k bcastW[k, m] * xflat[k, n]
    # xflat rows: 0 = A even-block pix, 1 = B even, 2 = A odd, 3 = B odd,
    #             4 = ones.
    # xB rows m: 0-47 -> even block (g=0), 64-111 -> odd block (g=1);
    # within a 48-group: r = b*24 + k.  k=0 is the bias/ones row.
    bcastW5 = np.zeros((5, 112), f32)
    for m in range(112):
        if 48 <= m < 64:
            continue
        g, r = (0, m) if m < 48 else (1, m - 64)
        b_loc, k = divmod(r, 24)
        if k == 0:
            bcastW5[4, m] = 1.0
        else:
            bcastW5[2 * g + b_loc, m] = 1.0
            bcastW5[4, m] = bf_s
    c["bcastW"] = bcastW5

    # Threshold column for the fused (x > thr) * x op.
    thrcol = np.zeros((112, 1), f32)
    for m in range(112):
        if 48 <= m < 64:
            thrcol[m, 0] = 1e30
            continue
        r = m if m < 48 else m - 64
        k = r % 24
        thrcol[m, 0] = -1e30 if k == 0 else f32(k * (k + 1) / 600.0)
    c["thrcol"] = thrcol

    # Block-diagonal [bf2 | wf2] weights for the decomposition matmul.
    # rows (b, k) at bases 0 and 64; cols m = b*64 + c.
    wbd = np.zeros((112, 128), f32)
    for base in (0, 64):
        for b_loc in range(2):
            for k in range(24):
                row = base + 24 * b_loc + k
                cols = slice(64 * b_loc, 64 * b_loc + 64)
                wbd[row, cols] = bf2.astype(f32) if k == 0 else wf2[:, k - 1].astype(f32)
    # Pack small constants into two [128, N] arrays (one SBUF tile each).
    packA = np.zeros((128, 96), f32)
    packA[0:64, 0:16] = c.pop("w1Ts")
    packA[0:16, 16:80] = c.pop("w2T")
    packA[0:16, 80:81] = c.pop("b1c")
    packA[0:64, 81:82] = c.pop("b2c")
    packA[0:64, 82:83] = c.pop("wfcol")
    packA[:, 83:84] = 1.0                      # ones128
    packA[0:112, 84:85] = thrcol
    c.pop("thrcol")
    packB = np.zeros((128, 368), f32)
    packB[0:112, 0:128] = wbd
    packB[:, 128:256] = np.eye(128, dtype=f32)
    packB[0:5, 256:368] = c.pop("bcastW")
    c["packA"] = packA
    c["packB"] = packB
    c["ones8k"] = np.ones((1, 8192), f32)
    return c


CONST_SHAPES = {
    "packA": (128, 96), "packB": (128, 368), "ones8k": (1, 8192),
}


def _build_nc(reps=1):
    from contextlib import ExitStack, nullcontext

    import concourse.bass as bass
    import concourse.bacc as bacc
    import concourse.tile as tile
    from concourse import mybir

    f32 = mybir.dt.float32
    Alu = mybir.AluOpType
    Act = mybir.ActivationFunctionType

    nc = bacc.Bacc("TRN2", target_bir_lowering=False, debug=False)
    feat = nc.dram_tensor("feat", [BL, C, T, F], f32, kind="ExternalInput")
    outp = nc.dram_tensor("outp", [BL, C, T, F], f32, kind="ExternalOutput")
    cts = {
        name: nc.dram_tensor(name, list(shape), f32, kind="ExternalInput")
        for name, shape in CONST_SHAPES.items()
    }

    with tile.TileContext(nc) as tc, ExitStack() as ctx:
        cpool = ctx.enter_context(tc.tile_pool(name="consts", bufs=1))
        cA = cpool.tile([128, 96], f32, tag="packA", name="c_packA")
        nc.gpsimd.dma_start(out=cA[:], in_=cts["packA"][:])
        cB = cpool.tile([128, 368], f32, tag="packB", name="c_packB")
        nc.gpsimd.dma_start(out=cB[:], in_=cts["packB"][:])
        sb = {
            "w1Ts": cA[0:64, 0:16], "w2T": cA[0:16, 16:80],
            "b1c": cA[0:16, 80:81], "b2c": cA[0:64, 81:82],
            "wfcol": cA[0:64, 82:83], "ones128": cA[:, 83:84],
            "thrcol": cA[0:112, 84:85],
            "wbd": cB[0:112, 0:128], "ident128": cB[:, 128:256],
            "bcastW": cB[0:5, 256:368],
        }

        loop_cm = tc.For_i(0, reps, 1) if reps > 1 else nullcontext()
        ctx.enter_context(loop_cm)
        persist = ctx.enter_context(tc.tile_pool(name="persist", bufs=1))
        p1pool = ctx.enter_context(tc.tile_pool(name="p1feat", bufs=3))

        small64 = persist.tile([64, 4096], f32, tag="small64", name="small64")
        def w64(idx, rows=64):
            return small64[0:rows, 256 * idx : 256 * idx + 256]
        ztsum = [w64(0), w64(1)]
        zfsum = [w64(2), w64(3)]
        zfpart_all = persist.tile([128, 256], f32, tag="zfpart", name="zfpart")
        zfpart = [
            [zfpart_all[:, 64 * (2 * b + h) : 64 * (2 * b + h) + 64]
             for h in range(2)]
            for b in range(BL)
        ]
        x128 = persist.tile([128, 1024], f32, tag="x128", name="x128")
        x_sb = [
            [x128[:, 256 * (2 * b + m) : 256 * (2 * b + m) + 256]
             for m in range(2)]
            for b in range(BL)
        ]

        # ---------------- Pass 1: row/col sums ----------------
        # 2 MB input tiles (16 channels); T-sums accumulate in [1, 2048]
        # PSUM rows (8 channels each), then ACT-copy to an SBUF row and
        # DMA-redistribute to [8, 256].
        with tc.tile_pool(name="ps_tsum", bufs=2, space="PSUM") as ppt:
            for b in range(BL):
                for q16 in range(C // 16):  # 16-channel groups
                    fts = []
                    for h in range(2):
                        ft = p1pool.tile([128, 16, F], f32, tag="ft", name="ft")
                        src = feat[b, 16 * q16 : 16 * q16 + 16,
                                   128 * h : 128 * h + 128, :]
                        eng = nc.sync if (q16 + h) % 2 == 0 else nc.scalar
                        eng.dma_start(out=ft[:], in_=src.transpose([1, 0, 2]))
                        nc.vector.tensor_reduce(
                            out=zfpart[b][h][:, 16 * q16 : 16 * q16 + 16],
                            in_=ft[:],
                            axis=mybir.AxisListType.X,
                            op=Alu.add,
                        )
                        fts.append(ft)
                    for half in range(2):  # two channel-octets
                        jj = 2 * q16 + half
                        tsum = ppt.tile([1, 2048], f32, tag="tsum", name="tsum")
                        for s in range(4):
                            for h in range(2):
                                nc.tensor.matmul(
                                    tsum[:, 512 * s : 512 * s + 512],
                                    sb["ones128"],
                                    fts[h][:, 8 * half + 2 * s : 8 * half + 2 * s + 2, :],
                                    start=(h == 0), stop=(h == 1),
                                )
                        ztrow = persist.tile([1, 2048], f32, tag="ztrow",
                                             name="ztrow", bufs=2)
                        nc.scalar.copy(ztrow[:], tsum[:])
                        nc.gpsimd.dma_start(
                            out=ztsum[b][8 * jj : 8 * jj + 8, :], in_=ztrow[:]
                        )

        with tc.tile_pool(name="ps_tp", bufs=2, space="PSUM") as pptp:
            for b in range(BL):
                for h in range(2):
                    tp = pptp.tile([C, 128], f32, tag="tp")
                    nc.tensor.transpose(tp[:], zfpart[b][h], sb["ident128"])
                    nc.scalar.copy(zfsum[b][:, 128 * h : 128 * h + 128], tp[:])

        # ---------------- SE branches + out1 ----------------
        def se_branch(zin, sidx):
            h1p = ppse.tile([OC, 256], f32, tag="h1p")
            nc.tensor.matmul(h1p[:], sb["w1Ts"], zin)
            h1s = small64[0:OC, 256 * (10 + sidx) : 256 * (10 + sidx) + 256]
            nc.scalar.activation(h1s, h1p[:], Act.Relu,
                                 bias=sb["b1c"], scale=1.0)
            h2p = ppse.tile([C, 256], f32, tag="h2p")
            nc.tensor.matmul(h2p[:], sb["w2T"], h1s)
            zout = w64(4 + sidx)
            nc.scalar.activation(zout, h2p[:], Act.Sigmoid,
                                 bias=sb["b2c"], scale=1.0)
            return zout

        with tc.tile_pool(name="ps_se", bufs=1, space="PSUM") as ppse:
            for b in range(BL):
                zt = se_branch(ztsum[b], 2 * b)
                zf = se_branch(zfsum[b], 2 * b + 1)
                wfzf = w64(8 + b)
                nc.vector.tensor_scalar_mul(wfzf, zf, sb["wfcol"])
                for m in range(2):
                    o1 = ppse.tile([128, F], f32, tag="o1")
                    nc.tensor.matmul(
                        o1[:], wfzf[:, 128 * m : 128 * m + 128], zt
                    )
                    nc.scalar.copy(x_sb[b][m], o1[:])

        # ---------------- x_flat: [5, 8192] per quarter ----------------
        # Quarter q covers pairs 16q..16q+15 (t-rows 64q..64q+63).  Row
        # layout: 0 = A even blocks, 1 = B even, 2 = A odd, 3 = B odd,
        # 4 = ones.  Even block of pair p = t-rows {4p, 4p+1}; odd =
        # {4p+2, 4p+3}.
        xfpool = ctx.enter_context(tc.tile_pool(name="xflat", bufs=2))
        xflat = []
        for q in range(4):
            xf = xfpool.tile([5, 8192], f32, tag="xf", name=f"xf{q}")
            m, tbase = divmod(q, 2)  # x_sb half-tile and row base (0/64)
            for par, (b_loc, off) in enumerate(
                [(0, 0), (1, 0), (0, 2), (1, 2)]
            ):
                srct = x_sb[b_loc][m]
                pitch = srct.ap[0][0]
                for sub in range(2):
                    row0 = 64 * tbase + off + sub
                    s0 = srct[row0 : row0 + 1, :]
                    src_ap = bass.AP(
                        tensor=s0.tensor, offset=s0.offset,
                        ap=[[4 * pitch, 16], [1, 256]],
                    )
                    d0 = xf[par : par + 1, :]
                    dst_ap = bass.AP(
                        tensor=d0.tensor, offset=d0.offset + 256 * sub,
                        ap=[[8192, 1], [512, 16], [1, 256]],
                    )
                    nc.gpsimd.dma_start(out=dst_ap, in_=src_ap)
            nc.gpsimd.dma_start(out=xf[4:5, :], in_=cts["ones8k"][:])
            xflat.append(xf)

        # ---------------- Pass 2 ----------------
        p2pool = ctx.enter_context(tc.tile_pool(name="p2feat", bufs=3))
        opool = ctx.enter_context(tc.tile_pool(name="outs", bufs=3))
        xbspool = ctx.enter_context(tc.tile_pool(name="xbs", bufs=2))
        dcpool = ctx.enter_context(tc.tile_pool(name="dcomp", bufs=2))
        ppxb = ctx.enter_context(tc.tile_pool(name="ps_xb", bufs=2, space="PSUM"))
        ppg = ctx.enter_context(tc.tile_pool(name="ps_g", bufs=2, space="PSUM"))

        # Two pairs (8 t-rows, 1 MB) per input/output DMA.
        for grp in range(NPAIR // 2):
            ft2 = p2pool.tile([128, 8, F], f32, tag="ft2", name="ft2")
            eng = nc.sync if grp % 2 == 0 else nc.scalar
            eng.dma_start(out=ft2[:], in_=feat[:, :, 8 * grp : 8 * grp + 8, :])
            ot = opool.tile([128, 8, F], f32, tag="ot", name="ot")
            for ii in range(2):
                i = 2 * grp + ii
                q, r = divmod(i, 16)
                xB = ppxb.tile([112, 512], f32, tag="xB")
                nc.tensor.matmul(
                    xB[:], sb["bcastW"], xflat[q][:, 512 * r : 512 * r + 512]
                )
                xBs = xbspool.tile([112, 512], f32, tag="xBs")
                nc.scalar.copy(xBs[:], xB[:])
                dc = dcpool.tile([112, 512], f32, tag="dc")
                nc.vector.scalar_tensor_tensor(
                    out=dc[:], in0=xBs[:], scalar=sb["thrcol"], in1=xB[:],
                    op0=Alu.is_gt, op1=Alu.mult,
                )
                gp = ppg.tile([128, 1024], f32, tag="gp")
                for g in (0, 1):
                    nc.tensor.matmul(
                        gp[:, 512 * g : 512 * g + 512],
                        sb["wbd"][64 * g : 64 * g + 48, :],
                        dc[64 * g : 64 * g + 48, :],
                    )
                nc.vector.tensor_tensor(
                    out=ot[:, 4 * ii : 4 * ii + 4, :],
                    in0=gp[:].rearrange("p (a b) -> p a b", a=4),
                    in1=ft2[:, 4 * ii : 4 * ii + 4, :], op=Alu.mult,
                )
            nc.gpsimd.dma_start(
                out=outp[:, :, 8 * grp : 8 * grp + 8, :], in_=ot[:]
            )

    nc.finalize()
    return nc


def _get_nc(reps=1):
    key = ("nc", reps)
    if key not in _CACHE:
        _CACHE[key] = _build_nc(reps)
    return _CACHE[key]


def _make_runner(nc, n_cores):
    """Cached jitted shard_map executor for `nc` (mirrors
    bass2jax.run_bass_via_pjrt but reusable across calls)."""
    import jax
    from jax.sharding import Mesh, PartitionSpec
    from jax.experimental.shard_map import shard_map
    from concourse import bass2jax, mybir

    bass2jax.install_neuronx_cc_hook()

    partition_name = (
        nc.partition_id_tensor.name if nc.partition_id_tensor else None
    )
    in_names, out_names, out_avals, zero_outs = [], [], [], []
    for alloc in nc.m.functions[0].allocations:
        if not isinstance(alloc, mybir.MemoryLocationSet):
            continue
        name = alloc.memorylocations[0].name
        if alloc.kind == "ExternalInput":
            if name != partition_name:
                in_names.append(name)
        elif alloc.kind == "ExternalOutput":
            out_names.append(name)
            shape = tuple(alloc.tensor_shape)
            dtype = mybir.dt.np(alloc.dtype)
            out_avals.append(jax.core.ShapedArray(shape, dtype))
            zero_outs.append(np.zeros(shape, dtype))
    n_params = len(in_names)
    all_in_names = in_names + out_names
    if partition_name is not None:
        all_in_names = all_in_names + [partition_name]
    donate = tuple(range(n_params, n_params + len(out_names)))

    def _body(*args):
        operands = list(args)
        if partition_name is not None:
            operands.append(bass2jax.partition_id_tensor())
        outs = bass2jax._bass_exec_p.bind(
            *operands,
            out_avals=tuple(out_avals),
            in_names=tuple(all_in_names),
            out_names=tuple(out_names),
            lowering_input_output_aliases=(),
            sim_require_finite=True,
            sim_require_nnan=True,
            nc=nc,
        )
        return tuple(outs)

    devices = jax.devices()[:n_cores]
    mesh = Mesh(np.asarray(devices), ("core",))
    specs = (PartitionSpec("core"),) * (n_params + len(out_names))
    sharded = jax.jit(
        shard_map(_body, mesh=mesh, in_specs=specs,
                  out_specs=(PartitionSpec("core"),) * len(out_names),
                  check_rep=False),
        donate_argnums=donate, keep_unused=True,
    )

    def run(in_maps):
        per_core = [[np.asarray(m[name]) for name in in_names] for m in in_maps]
        concat_in = [
            np.concatenate([per_core[c][i] for c in range(n_cores)], axis=0)
            for i in range(n_params)
        ]
        concat_zeros = [
            np.zeros((n_cores * z.shape[0], *z.shape[1:]), z.dtype)
            for z in zero_outs
        ]
        out_arrs = sharded(*concat_in, *concat_zeros)
        return [
            {
                name: np.asarray(out_arrs[i]).reshape(n_cores, *out_avals[i].shape)[c]
                for i, name in enumerate(out_names)
            }
            for c in range(n_cores)
        ]

    def make_chain(n_reps):
        """Jitted callable running the kernel n_reps times back-to-back on
        device (each rep's outputs become the next rep's output buffers),
        for overhead-free timing via slope."""
        def _bodyN(*args):
            ins = list(args[:n_params])
            outs = list(args[n_params:])
            for _ in range(n_reps):
                outs = list(_body(*ins, *outs))
            return tuple(outs)

        return jax.jit(
            shard_map(_bodyN, mesh=mesh, in_specs=specs,
                      out_specs=(PartitionSpec("core"),) * len(out_names),
                      check_rep=False),
            keep_unused=True,
        )

    run.sharded = sharded
    run.in_names = in_names
    run.out_names = out_names
    run.zero_outs = zero_outs
    run.n_params = n_params
    run.make_chain = make_chain
    return run


def _get_runner(reps=1):
    key = ("runner", reps)
    if key not in _CACHE:
        _CACHE[key] = _make_runner(_get_nc(reps), N_CORES)
    return _CACHE[key]


def kernel(**inputs):
    feature_in = np.ascontiguousarray(np.asarray(inputs["feature_in"], np.float32))
    consts = _host_constants(
        np.asarray(inputs["w1"]), np.asarray(inputs["b1"]),
        np.asarray(inputs["w2"]), np.asarray(inputs["b2"]),
        np.asarray(inputs["wf"]), np.asarray(inputs["bf"]),
        np.asarray(inputs["wf2"]), np.asarray(inputs["bf2"]),
    )
    in_maps = []
    for core in range(N_CORES):
        m = {"feat": feature_in[BL * core : BL * core + BL]}
        m.update(consts)
        in_maps.append(m)

    run = _get_runner()
    res = run(in_maps)
    out = np.concatenate([res[c]["outp"] for c in range(N_CORES)], axis=0)
    return out.reshape(B, C, T, F).astype(np.float32)


# revision 24
# speedup vs baseline: 1.0261x; 1.0261x over previous
"""Trainium2 Bass kernel for the DTFA (dual-attention SE + threshold
decomposition) module.

Math (per batch b):
  zt = SE(mean_T(x))            # [C, F]
  zf = SE(mean_F(x))            # [C, T]
  out1[t,f] = sum_c wf[c]*zf[c,t]*zt[c,f] + bf          (rank-C matmul)
  dcomp[k]  = where(out1 > thr_k, out1, 0), k=1..23
  out[c]    = (sum_k wf2[c,k]*dcomp[k] + bf2[c]) * x[c]

Sharding: pure data-parallel, 2 batches per core on 8 cores.

Pass 1 streams the input as [128t, 8c*256f] tiles (1 MB DMAs): T-sums via
PE ones-matmuls into [1, 2048] PSUM rows, F-sums via DVE tensor_reduce.
Pass 2 processes 1024-pixel block pairs (blocks 2i, 2i+1): a PE
broadcast-matmul replicates out1 into 2x(ones + 23 thresholds) x 2
batches rows ([112, 512] PSUM, bias folded via the ones row), one fused
DVE scalar_tensor_tensor forms (x > thr) * x, a block-diagonal [bf2|wf2]
matmul produces out2 for both batches at once, and a single DVE
tensor_tensor multiplies by the re-streamed input.
"""

import numpy as np

B, C, OC, T, F = 16, 64, 16, 256, 256
N_THR = 23
N_CORES = 8
BL = B // N_CORES  # local batches per core = 2
PIX = T * F        # 65536 per batch
NPAIR = 64         # pairs of adjacent 512-pix blocks (t-quads) per batch

_CACHE = {}


def _host_constants(w1, b1, w2, b2, wf, bf, wf2, bf2):
    f32 = np.float32
    c = {}
    # SE weights. lhsT layout [K, M]; fold the 1/256 mean scale into w1.
    c["w1Ts"] = np.ascontiguousarray(w1.T.astype(f32) / 256.0)          # [64, 16]
    c["w2T"] = np.ascontiguousarray(w2.T.astype(f32))                   # [16, 64]
    c["b1c"] = b1.astype(f32).reshape(OC, 1)
    c["b2c"] = b2.astype(f32).reshape(C, 1)
    c["wfcol"] = wf.astype(f32).reshape(C, 1)                            # [64, 1]
    bf_s = float(np.asarray(bf).reshape(-1)[0])

    # Broadcast matmul weights: xB[m, n] = sum_k bcastW[k, m] * xflat[k, n]
    # xflat rows: 0 = A even-block pix, 1 = B even, 2 = A odd, 3 = B odd,
    #             4 = ones.
    # xB rows m: 0-47 -> even block (g=0), 64-111 -> odd block (g=1);
    # within a 48-group: r = b*24 + k.  k=0 is the bias/ones row.
    bcastW5 = np.zeros((5, 112), f32)
    for m in range(112):
        if 48 <= m < 64:
            continue
        g, r = (0, m) if m < 48 else (1, m - 64)
        b_loc, k = divmod(r, 24)
        if k == 0:
            bcastW5[4, m] = 1.0
        else:
            bcastW5[2 * g + b_loc, m] = 1.0
            bcastW5[4, m] = bf_s
    c["bcastW"] = bcastW5

    # Threshold column for the fused (x > thr) * x op.
    thrcol = np.zeros((112, 1), f32)
    for m in range(112):
        if 48 <= m < 64:
            thrcol[m, 0] = 1e30
            continue
        r = m if m < 48 else m - 64
        k = r % 24
        thrcol[m, 0] = -1e30 if k == 0 else f32(k * (k + 1) / 600.0)
    c["thrcol"] = thrcol

    # Block-diagonal [bf2 | wf2] weights for the decomposition matmul.
    # rows (b, k) at bases 0 and 64; cols m = b*64 + c.
    wbd = np.zeros((112, 128), f32)
    for base in (0, 64):
        for b_loc in range(2):
            for k in range(24):
                row = base + 24 * b_loc + k
                cols = slice(64 * b_loc, 64 * b_loc + 64)
                wbd[row, cols] = bf2.astype(f32) if k == 0 else wf2[:, k - 1].astype(f32)
    # Pack small constants into two [128, N] arrays (one SBUF tile each).
    packA = np.zeros((128, 96), f32)
    packA[0:64, 0:16] = c.pop("w1Ts")
    packA[0:16, 16:80] = c.pop("w2T")
    packA[0:16, 80:81] = c.pop("b1c")
    packA[0:64, 81:82] = c.pop("b2c")
    packA[0:64, 82:83] = c.pop("wfcol")
    packA[:, 83:84] = 1.0                      # ones128
    packA[0:112, 84:85] = thrcol
    c.pop("thrcol")
    packB = np.zeros((128, 368), f32)
    packB[0:112, 0:128] = wbd
    packB[:, 128:256] = np.eye(128, dtype=f32)
    packB[0:5, 256:368] = c.pop("bcastW")
    c["packA"] = packA
    c["packB"] = packB
    c["ones8k"] = np.ones((1, 8192), f32)
    return c


CONST_SHAPES = {
    "packA": (128, 96), "packB": (128, 368), "ones8k": (1, 8192),
}


def _build_nc(reps=1):
    from contextlib import ExitStack, nullcontext

    import concourse.bass as bass
    import concourse.bacc as bacc
    import concourse.tile as tile
    from concourse import mybir

    f32 = mybir.dt.float32
    Alu = mybir.AluOpType
    Act = mybir.ActivationFunctionType

    nc = bacc.Bacc("TRN2", target_bir_lowering=False, debug=False)
    feat = nc.dram_tensor("feat", [BL, C, T, F], f32, kind="ExternalInput")
    outp = nc.dram_tensor("outp", [BL, C, T, F], f32, kind="ExternalOutput")
    cts = {
        name: nc.dram_tensor(name, list(shape), f32, kind="ExternalInput")
        for name, shape in CONST_SHAPES.items()
    }

    with tile.TileContext(nc) as tc, ExitStack() as ctx:
        cpool = ctx.enter_context(tc.tile_pool(name="consts", bufs=1))
        cA = cpool.tile([128, 96], f32, tag="packA", name="c_packA")
        nc.gpsimd.dma_start(out=cA[:], in_=cts["packA"][:])
        cB = cpool.tile([128, 368], f32, tag="packB", name="c_packB")
        nc.gpsimd.dma_start(out=cB[:], in_=cts["packB"][:])
        sb = {
            "w1Ts": cA[0:64, 0:16], "w2T": cA[0:16, 16:80],
            "b1c": cA[0:16, 80:81], "b2c": cA[0:64, 81:82],
            "wfcol": cA[0:64, 82:83], "ones128": cA[:, 83:84],
            "thrcol": cA[0:112, 84:85],
            "wbd": cB[0:112, 0:128], "ident128": cB[:, 128:256],
            "bcastW": cB[0:5, 256:368],
        }

        loop_cm = tc.For_i(0, reps, 1) if reps > 1 else nullcontext()
        ctx.enter_context(loop_cm)
        persist = ctx.enter_context(tc.tile_pool(name="persist", bufs=1))
        p1pool = ctx.enter_context(tc.tile_pool(name="p1feat", bufs=3))

        small64 = persist.tile([64, 4096], f32, tag="small64", name="small64")
        def w64(idx, rows=64):
            return small64[0:rows, 256 * idx : 256 * idx + 256]
        ztsum = [w64(0), w64(1)]
        zfsum = [w64(2), w64(3)]
        zfpart_all = persist.tile([128, 256], f32, tag="zfpart", name="zfpart")
        zfpart = [
            [zfpart_all[:, 64 * (2 * b + h) : 64 * (2 * b + h) + 64]
             for h in range(2)]
            for b in range(BL)
        ]
        x128 = persist.tile([128, 1024], f32, tag="x128", name="x128")
        x_sb = [
            [x128[:, 256 * (2 * b + m) : 256 * (2 * b + m) + 256]
             for m in range(2)]
            for b in range(BL)
        ]

        # ---------------- Pass 1: row/col sums ----------------
        # 2 MB input tiles (16 channels); T-sums accumulate in [1, 2048]
        # PSUM rows (8 channels each), then ACT-copy to an SBUF row and
        # DMA-redistribute to [8, 256].
        with tc.tile_pool(name="ps_tsum", bufs=2, space="PSUM") as ppt:
            for b in range(BL):
                for q16 in range(C // 16):  # 16-channel groups
                    fts = []
                    for h in range(2):
                        ft = p1pool.tile([128, 16, F], f32, tag="ft", name="ft")
                        src = feat[b, 16 * q16 : 16 * q16 + 16,
                                   128 * h : 128 * h + 128, :]
                        eng = nc.sync if (q16 + h) % 2 == 0 else nc.scalar
                        eng.dma_start(out=ft[:], in_=src.transpose([1, 0, 2]))
                        nc.vector.tensor_reduce(
                            out=zfpart[b][h][:, 16 * q16 : 16 * q16 + 16],
                            in_=ft[:],
                            axis=mybir.AxisListType.X,
                            op=Alu.add,
                        )
                        fts.append(ft)
                    for half in range(2):  # two channel-octets
                        jj = 2 * q16 + half
                        tsum = ppt.tile([1, 2048], f32, tag="tsum", name="tsum")
                        for s in range(4):
                            for h in range(2):
                                nc.tensor.matmul(
                                    tsum[:, 512 * s : 512 * s + 512],
                                    sb["ones128"],
                                    fts[h][:, 8 * half + 2 * s : 8 * half + 2 * s + 2, :],
                                    start=(h == 0), stop=(h == 1),
                                )
                        ztrow = persist.tile([1, 2048], f32, tag="ztrow",
                                             name="ztrow", bufs=2)
                        nc.scalar.copy(ztrow[:], tsum[:])
                        nc.gpsimd.dma_start(
                            out=ztsum[b][8 * jj : 8 * jj + 8, :], in_=ztrow[:]
                        )

        with tc.tile_pool(name="ps_tp", bufs=2, space="PSUM") as pptp:
            for b in range(BL):
                for h in range(2):
                    tp = pptp.tile([C, 128], f32, tag="tp")
                    nc.tensor.transpose(tp[:], zfpart[b][h], sb["ident128"])
                    nc.scalar.copy(zfsum[b][:, 128 * h : 128 * h + 128], tp[:])

        # ---------------- SE branches + out1 ----------------
        def se_branch(zin, sidx):
            h1p = ppse.tile([OC, 256], f32, tag="h1p")
            nc.tensor.matmul(h1p[:], sb["w1Ts"], zin)
            h1s = small64[0:OC, 256 * (10 + sidx) : 256 * (10 + sidx) + 256]
            nc.scalar.activation(h1s, h1p[:], Act.Relu,
                                 bias=sb["b1c"], scale=1.0)
            h2p = ppse.tile([C, 256], f32, tag="h2p")
            nc.tensor.matmul(h2p[:], sb["w2T"], h1s)
            zout = w64(4 + sidx)
            nc.scalar.activation(zout, h2p[:], Act.Sigmoid,
                                 bias=sb["b2c"], scale=1.0)
            return zout

        with tc.tile_pool(name="ps_se", bufs=1, space="PSUM") as ppse:
            for b in range(BL):
                zt = se_branch(ztsum[b], 2 * b)
                zf = se_branch(zfsum[b], 2 * b + 1)
                wfzf = w64(8 + b)
                nc.vector.tensor_scalar_mul(wfzf, zf, sb["wfcol"])
                for m in range(2):
                    o1 = ppse.tile([128, F], f32, tag="o1")
                    nc.tensor.matmul(
                        o1[:], wfzf[:, 128 * m : 128 * m + 128], zt
                    )
                    nc.scalar.copy(x_sb[b][m], o1[:])

        # ---------------- x_flat: [5, 8192] per quarter ----------------
        # Quarter q covers pairs 16q..16q+15 (t-rows 64q..64q+63).  Row
        # layout: 0 = A even blocks, 1 = B even, 2 = A odd, 3 = B odd,
        # 4 = ones.  Even block of pair p = t-rows {4p, 4p+1}; odd =
        # {4p+2, 4p+3}.
        xfpool = ctx.enter_context(tc.tile_pool(name="xflat", bufs=2))
        xflat = []
        for q in range(8):  # groups of 8 pairs (32 t-rows)
            xf = xfpool.tile([5, 4096], f32, tag="xf", name=f"xf{q}")
            m, tbase = divmod(q, 4)  # x_sb half-tile and 32-row base
            for par, (b_loc, off) in enumerate(
                [(0, 0), (1, 0), (0, 2), (1, 2)]
            ):
                srct = x_sb[b_loc][m]
                pitch = srct.ap[0][0]
                for sub in range(2):
                    row0 = 32 * tbase + off + sub
                    s0 = srct[row0 : row0 + 1, :]
                    src_ap = bass.AP(
                        tensor=s0.tensor, offset=s0.offset,
                        ap=[[4 * pitch, 8], [1, 256]],
                    )
                    d0 = xf[par : par + 1, :]
                    dst_ap = bass.AP(
                        tensor=d0.tensor, offset=d0.offset + 256 * sub,
                        ap=[[4096, 1], [512, 8], [1, 256]],
                    )
                    nc.gpsimd.dma_start(out=dst_ap, in_=src_ap)
            nc.gpsimd.dma_start(out=xf[4:5, :], in_=cts["ones8k"][0:1, 0:4096])
            xflat.append(xf)

        # ---------------- Pass 2 ----------------
        p2pool = ctx.enter_context(tc.tile_pool(name="p2feat", bufs=3))
        opool = ctx.enter_context(tc.tile_pool(name="outs", bufs=3))
        xbspool = ctx.enter_context(tc.tile_pool(name="xbs", bufs=2))
        dcpool = ctx.enter_context(tc.tile_pool(name="dcomp", bufs=2))
        ppxb = ctx.enter_context(tc.tile_pool(name="ps_xb", bufs=2, space="PSUM"))
        ppg = ctx.enter_context(tc.tile_pool(name="ps_g", bufs=2, space="PSUM"))

        # Two pairs (8 t-rows, 1 MB) per input/output DMA.
        for grp in range(NPAIR // 2):
            ft2 = p2pool.tile([128, 8, F], f32, tag="ft2", name="ft2")
            nc.sync.dma_start(out=ft2[:], in_=feat[:, :, 8 * grp : 8 * grp + 8, :])
            ot = opool.tile([128, 8, F], f32, tag="ot", name="ot")
            for ii in range(2):
                i = 2 * grp + ii
                q, r = divmod(i, 8)
                xB = ppxb.tile([112, 512], f32, tag="xB")
                nc.tensor.matmul(
                    xB[:], sb["bcastW"], xflat[q][:, 512 * r : 512 * r + 512]
                )
                xBs = xbspool.tile([112, 512], f32, tag="xBs")
                nc.scalar.copy(xBs[:], xB[:])
                dc = dcpool.tile([112, 512], f32, tag="dc")
                nc.vector.scalar_tensor_tensor(
                    out=dc[:], in0=xBs[:], scalar=sb["thrcol"], in1=xB[:],
                    op0=Alu.is_gt, op1=Alu.mult,
                )
                gp = ppg.tile([128, 1024], f32, tag="gp")
                for g in (0, 1):
                    nc.tensor.matmul(
                        gp[:, 512 * g : 512 * g + 512],
                        sb["wbd"][64 * g : 64 * g + 48, :],
                        dc[64 * g : 64 * g + 48, :],
                    )
                nc.vector.tensor_tensor(
                    out=ot[:, 4 * ii : 4 * ii + 4, :],
                    in0=gp[:].rearrange("p (a b) -> p a b", a=4),
                    in1=ft2[:, 4 * ii : 4 * ii + 4, :], op=Alu.mult,
                )
            nc.scalar.dma_start(
                out=outp[:, :, 8 * grp : 8 * grp + 8, :], in_=ot[:]
            )

    nc.finalize()
    return nc


def _get_nc(reps=1):
    key = ("nc", reps)
    if key not in _CACHE:
        _CACHE[key] = _build_nc(reps)
    return _CACHE[key]


def _make_runner(nc, n_cores):
    """Cached jitted shard_map executor for `nc` (mirrors
    bass2jax.run_bass_via_pjrt but reusable across calls)."""
    import jax
    from jax.sharding import Mesh, PartitionSpec
    from jax.experimental.shard_map import shard_map
    from concourse import bass2jax, mybir

    bass2jax.install_neuronx_cc_hook()

    partition_name = (
        nc.partition_id_tensor.name if nc.partition_id_tensor else None
    )
    in_names, out_names, out_avals, zero_outs = [], [], [], []
    for alloc in nc.m.functions[0].allocations:
        if not isinstance(alloc, mybir.MemoryLocationSet):
            continue
        name = alloc.memorylocations[0].name
        if alloc.kind == "ExternalInput":
            if name != partition_name:
                in_names.append(name)
        elif alloc.kind == "ExternalOutput":
            out_names.append(name)
            shape = tuple(alloc.tensor_shape)
            dtype = mybir.dt.np(alloc.dtype)
            out_avals.append(jax.core.ShapedArray(shape, dtype))
            zero_outs.append(np.zeros(shape, dtype))
    n_params = len(in_names)
    all_in_names = in_names + out_names
    if partition_name is not None:
        all_in_names = all_in_names + [partition_name]
    donate = tuple(range(n_params, n_params + len(out_names)))

    def _body(*args):
        operands = list(args)
        if partition_name is not None:
            operands.append(bass2jax.partition_id_tensor())
        outs = bass2jax._bass_exec_p.bind(
            *operands,
            out_avals=tuple(out_avals),
            in_names=tuple(all_in_names),
            out_names=tuple(out_names),
            lowering_input_output_aliases=(),
            sim_require_finite=True,
            sim_require_nnan=True,
            nc=nc,
        )
        return tuple(outs)

    devices = jax.devices()[:n_cores]
    mesh = Mesh(np.asarray(devices), ("core",))
    specs = (PartitionSpec("core"),) * (n_params + len(out_names))
    sharded = jax.jit(
        shard_map(_body, mesh=mesh, in_specs=specs,
                  out_specs=(PartitionSpec("core"),) * len(out_names),
                  check_rep=False),
        donate_argnums=donate, keep_unused=True,
    )

    def run(in_maps):
        per_core = [[np.asarray(m[name]) for name in in_names] for m in in_maps]
        concat_in = [
            np.concatenate([per_core[c][i] for c in range(n_cores)], axis=0)
            for i in range(n_params)
        ]
        concat_zeros = [
            np.zeros((n_cores * z.shape[0], *z.shape[1:]), z.dtype)
            for z in zero_outs
        ]
        out_arrs = sharded(*concat_in, *concat_zeros)
        return [
            {
                name: np.asarray(out_arrs[i]).reshape(n_cores, *out_avals[i].shape)[c]
                for i, name in enumerate(out_names)
            }
            for c in range(n_cores)
        ]

    def make_chain(n_reps):
        """Jitted callable running the kernel n_reps times back-to-back on
        device (each rep's outputs become the next rep's output buffers),
        for overhead-free timing via slope."""
        def _bodyN(*args):
            ins = list(args[:n_params])
            outs = list(args[n_params:])
            for _ in range(n_reps):
                outs = list(_body(*ins, *outs))
            return tuple(outs)

        return jax.jit(
            shard_map(_bodyN, mesh=mesh, in_specs=specs,
                      out_specs=(PartitionSpec("core"),) * len(out_names),
                      check_rep=False),
            keep_unused=True,
        )

    run.sharded = sharded
    run.in_names = in_names
    run.out_names = out_names
    run.zero_outs = zero_outs
    run.n_params = n_params
    run.make_chain = make_chain
    return run


def _get_runner(reps=1):
    key = ("runner", reps)
    if key not in _CACHE:
        _CACHE[key] = _make_runner(_get_nc(reps), N_CORES)
    return _CACHE[key]


def kernel(**inputs):
    feature_in = np.ascontiguousarray(np.asarray(inputs["feature_in"], np.float32))
    consts = _host_constants(
        np.asarray(inputs["w1"]), np.asarray(inputs["b1"]),
        np.asarray(inputs["w2"]), np.asarray(inputs["b2"]),
        np.asarray(inputs["wf"]), np.asarray(inputs["bf"]),
        np.asarray(inputs["wf2"]), np.asarray(inputs["bf2"]),
    )
    in_maps = []
    for core in range(N_CORES):
        m = {"feat": feature_in[BL * core : BL * core + BL]}
        m.update(consts)
        in_maps.append(m)

    run = _get_runner()
    res = run(in_maps)
    out = np.concatenate([res[c]["outp"] for c in range(N_CORES)], axis=0)
    return out.reshape(B, C, T, F).astype(np.float32)
